# revision 16
# baseline (speedup 1.0000x reference)
"""Causal self-attention kernel for Trainium2, 8 NeuronCores, data-parallel over batch.

Problem: B=4096 independent attentions, T=64, DIM=128, 4 heads of 32; y = proj(attn(x)).
k_in / v_in inputs are unused by the module (overwritten internally) -> never shipped.

v3 "chunk-dense bf16, instruction-minimized" (per core: 512 batches = 32768
tokens, 64 mega-tiles of 512 tokens = 4 chunks of 128 tokens = 2 batches each):
  - All matmuls bf16 (fp32 runs as 2 half-speed passes); x is cast to bf16 on
    the idle GpSimd engine so even the PE transposes run at bf16 rate.
  - q^T/k^T full-width [128,512] single matmuls. q-bias lands in the 4 strided
    PSUM->SBUF copies that build the zero-padded qt2 tiles; k-bias dropped
    (softmax-invariant); v-bias folded into the proj bias on host.
  - scores K=64 pair-packed, chunk-dense, TRANSPOSED: per (head-pair X, chunk
    c) one [64,128]x[64,256] -> [128,256] matmul. qt2[(X, hr', d), (c, hr,
    b*64+qq)] holds q with zeros on hr'!=hr (zeros written ONCE into
    persistent tiles), so the pair contraction picks out head 2X+hr. The two
    batches inside a chunk are separated by the mask seed (-80 on cross-batch
    and causal-invalid pairs): exp() kills them. attn col = 1024X + 256c +
    128hr + (b*64+qq); PSUM bank = (X, c-pair); each bank's accumulation
    group = (identity-matmul mask seed + 2 scorers).
  - softmax denominators: 4 accumulating indicator matmuls -> su[4,512],
    reciprocal_approx_fast (18 bits), bf16 cast, broadcast to rbP[128,512]
    (row = feature) by 2 matmuls with out bases 0/64, one ACT copy -> SBUF.
  - attn@v UNNORMALIZED (normalization commutes with the key-sum), pair-merged:
    per (c, X) one [128,64]x[128,256] matmul -> ytP[64X+(hr,d), (c,hr',bqq)];
    cross-head (hr!=hr') halves are garbage, never read; cross-batch terms
    are ~0 through the masked attn. 4 strided DVE muls extract the valid
    blocks, normalize by rbS, and assemble yT[128,512] (fin-major) in SBUF.
  - proj natural: per chunk one K=128 matmul (lhsT = yT chunk, moving = Wp),
    bias via DVE tensor_add; proj of mega m-1 is emitted inside mega m
    (software pipelining) so the PE never waits on the DVE normalize.
"""

import sys

for _p in ("/opt/trn_rl_repo", "/root/.axon_site/_ro/trn_rl_repo"):
    if _p not in sys.path:
        sys.path.insert(0, _p)

from contextlib import ExitStack

import numpy as np
import ml_dtypes

import concourse.bass as bass
import concourse.tile as tile
from concourse import bacc
from concourse import mybir
from concourse.bass_utils import run_bass_kernel_spmd

F32 = mybir.dt.float32
BF16 = mybir.dt.bfloat16
NP_BF16 = ml_dtypes.bfloat16

B, T, D, H, HS = 4096, 64, 128, 4, 32
NCORES = 8
BC = B // NCORES            # 512 batches per core
TOK = BC * T                # 32768 tokens per core
MEGA = 512                  # tokens per mega-tile (4 chunks of 128 = 8 batches)
NMEGA = TOK // MEGA         # 64
SCALE = 1.0 / float(np.sqrt(HS))
NEG = -80.0                 # additive causal mask (exp(-80) ~ 1.8e-35, harmless)

_CACHE = {}
LAST_RESULT = None


def _host_consts(W_attn, b_attn, W_proj, b_proj):
    """Precompute constant tiles on host: one fp32 pack and one bf16 pack."""
    Wq = np.ascontiguousarray(W_attn[:, 0:128] * SCALE)          # [128,128]
    Wk = np.ascontiguousarray(W_attn[:, 128:256])
    Wv = np.ascontiguousarray(W_attn[:, 256:384])
    bqs = (b_attn[0:128] * SCALE).reshape(128, 1)                # [128,1]
    bv = b_attn[256:384]
    bp_eff = b_proj + bv @ W_proj                                # [128]
    biasP = np.ascontiguousarray(
        np.broadcast_to(bp_eff.reshape(1, 1, 128), (128, 4, 128)).reshape(128, 512)
    )
    ident = np.eye(128, dtype=np.float32)
    # mask, transposed chunk-dense: rows = chunk token (b*64+kk); 512 cols of
    # 4 repeats of (b'*64+qq); 0 iff same batch and kk<=qq else NEG
    bkk = np.arange(128).reshape(128, 1)
    bqq = np.arange(128).reshape(1, 128)
    ok = ((bkk // 64) == (bqq // 64)) & ((bkk % 64) <= (bqq % 64))
    m128 = np.where(ok, 0.0, NEG).astype(np.float32)             # [128,128]
    maskT = np.ascontiguousarray(np.tile(m128, (1, 4)))          # [128,512]
    # sums stationary: sumsI[:, 4h+j] = (j==h)
    sumsI = np.zeros((128, 16), dtype=np.float32)
    for h in range(4):
        sumsI[:, 4 * h + h] = 1.0
    # bcast stationary: rb row (X, hr, d) <- rec row h=2X+hr. Two matmuls
    # (out bases 0/64): S4b[h, 64X+32hr+d] = 1, col-halves sliced per X.
    S4b = np.zeros((128, 128), dtype=np.float32)
    for h in range(4):
        c0 = 64 * (h // 2) + 32 * (h % 2)
        S4b[h, c0:c0 + 32] = 1.0

    f32_parts = [
        ("bq", bqs.astype(np.float32)),
        ("biasP", biasP.astype(np.float32)),
    ]
    bf_parts = [
        ("wq", Wq), ("wk", Wk), ("wv", Wv),
        ("wp", np.ascontiguousarray(W_proj)),
        ("identb", ident),
        ("maskT", maskT),
        ("sumsI", sumsI),
        ("S4b", S4b),
    ]

    def pack(parts, npdt):
        arrs = [np.asarray(a, dtype=np.float32).astype(npdt) for _, a in parts]
        packed = np.concatenate(arrs, axis=1)
        offs, off = {}, 0
        for (name, _), a in zip(parts, arrs):
            offs[name] = (off, a.shape[1])
            off += a.shape[1]
        return np.ascontiguousarray(packed), offs

    cf, cf_offs = pack(f32_parts, np.float32)
    cb, cb_offs = pack(bf_parts, NP_BF16)
    return cf, cf_offs, cb, cb_offs


def _build_program(cf_offs, cf_cols, cb_offs, cb_cols, ntok=TOK, stage=9):
    nmega = ntok // MEGA
    nc = bacc.Bacc()
    x_p = nc.declare_dram_parameter("x", [ntok, D], F32, isOutput=False)
    y_p = nc.declare_dram_parameter("y", [ntok, D], F32, isOutput=True)
    cf_p = nc.declare_dram_parameter("cf", [128, cf_cols], F32, isOutput=False)
    cb_p = nc.declare_dram_parameter("cb", [128, cb_cols], BF16, isOutput=False)

    # token index: t = m*512 + c*128 + p  (c = chunk, p = partition = b*64+qq)
    x_v = x_p.rearrange("(m c p) d -> m p c d", c=4, p=128)
    y_v = y_p.rearrange("(m c p) d -> m p c d", c=4, p=128)

    Copy = mybir.ActivationFunctionType.Copy
    Exp = mybir.ActivationFunctionType.Exp

    with tile.TileContext(nc) as tc, ExitStack() as ctx:
        cpool = ctx.enter_context(tc.tile_pool(name="consts", bufs=1))
        sb = ctx.enter_context(tc.tile_pool(name="sb", bufs=2))
        psA = ctx.enter_context(tc.tile_pool(name="psA", bufs=2, space="PSUM"))
        psB = ctx.enter_context(tc.tile_pool(name="psB", bufs=1, space="PSUM"))
        psC = ctx.enter_context(tc.tile_pool(name="psC", bufs=1, space="PSUM"))

        callf = cpool.tile([128, cf_cols], F32, tag="c_f32")
        nc.sync.dma_start(out=callf[:], in_=cf_p[:])
        callb = cpool.tile([128, cb_cols], BF16, tag="c_bf")
        nc.sync.dma_start(out=callb[:], in_=cb_p[:])
        ctf = {n: callf[:, o:o + w] for n, (o, w) in cf_offs.items()}
        ctb = {n: callb[:, o:o + w] for n, (o, w) in cb_offs.items()}

        # persistent zero-padded q tiles for K=64 pair-packed scorers:
        # qt2[ph][(X, hr', d) row, (c, hr, bqq) col]; hr'!=hr blocks stay 0
        # (written once here), so the pair contraction picks out head 2X+hr.
        qt2 = [cpool.tile([128, 4, 2, 128], BF16, tag=f"qt2_{ph}",
                          name=f"qt2_{ph}") for ph in range(2)]
        for ph in range(2):
            for X in range(2):
                nc.vector.memset(qt2[ph][64 * X + 32:64 * X + 64, :, 0, :], 0.0)
                nc.vector.memset(qt2[ph][64 * X:64 * X + 32, :, 1, :], 0.0)

        # software-pipeline state: proj+bias+DMA of mega m-1 emitted inside m
        pend = {}

        def emit_proj(p):
            yT, m = p["yT"], p["m"]
            yf = psA.tile([128, 512], F32, tag="mm512")
            for c in range(4):
                nc.tensor.matmul(
                    yf[:, c * 128:(c + 1) * 128],
                    yT[:, c * 128:(c + 1) * 128],
                    ctb["wp"],
                    start=True, stop=True,
                )
            y_out = sb.tile([128, 512], F32, tag="y_out")
            nc.vector.tensor_add(y_out[:], yf[:], ctf["biasP"])
            nc.sync.dma_start(
                out=y_v[m], in_=y_out[:].rearrange("p (c d) -> p c d", c=4)
            )

        for m in range(nmega):
            # ---- load x natural [p, c, d]; cast to bf16 on the idle GpSimd
            x_nat = sb.tile([128, 4, 128], F32, tag="x_nat")
            nc.sync.dma_start(out=x_nat[:], in_=x_v[m])
            xb = sb.tile([128, 4, 128], BF16, tag="xb")
            nc.gpsimd.tensor_copy(xb[:], x_nat[:])

            # ---- x^T via PE transpose (bf16)
            xT_ps = psA.tile([128, 512], BF16, tag="mm512")
            for c in range(4):
                nc.tensor.transpose(
                    xT_ps[:, c * 128:(c + 1) * 128], xb[:, c, :], ctb["identb"]
                )
            xT = sb.tile([128, 512], BF16, tag="xT")
            nc.scalar.activation(xT[:], xT_ps[:], Copy)

            # ---- k^T, v, q^T full-width (order chosen for PSUM slot reuse)
            k_ps = psA.tile([128, 512], F32, tag="mm512")
            nc.tensor.matmul(k_ps[:], ctb["wk"], xT[:], start=True, stop=True)
            ktF = sb.tile([128, 512], BF16, tag="ktF")
            nc.scalar.activation(ktF[:], k_ps[:], Copy)

            v_ps = psA.tile([128, 512], F32, tag="mm512")
            for c in range(4):
                nc.tensor.matmul(
                    v_ps[:, c * 128:(c + 1) * 128],
                    xT[:, c * 128:(c + 1) * 128],
                    ctb["wv"],
                    start=True, stop=True,
                )
            v_s = sb.tile([128, 512], BF16, tag="v_s")
            nc.scalar.activation(v_s[:], v_ps[:], Copy)

            q_ps = psA.tile([128, 512], F32, tag="mm512")
            nc.tensor.matmul(q_ps[:], ctb["wq"], xT[:], start=True, stop=True)
            q2 = qt2[m % 2]
            qpv = q_ps[:].rearrange("p (c q) -> p c q", c=4)
            for X in range(2):
                for hr in range(2):
                    r0 = 64 * X + 32 * hr
                    nc.vector.tensor_scalar_add(
                        q2[r0:r0 + 32, :, hr, :], qpv[r0:r0 + 32],
                        ctf["bq"][r0:r0 + 32, 0:1],
                    )

            # ---- proj of previous mega (PE fill while DVE works)
            if pend:
                emit_proj(pend)
                pend = {}

            # ---- scores, K=64 pair-packed, chunk-dense, transposed
            # attn col = 1024X + 256c + 128hr + bqq; bank = (X, c-pair)
            sc = psB.tile([128, 2048], F32, tag="sc")
            attn_u = sb.tile([128, 2048], BF16, tag="attn")
            for X in range(2):
                for cp in range(2):
                    nc.tensor.matmul(
                        sc[:, 1024 * X + 512 * cp:1024 * X + 512 * (cp + 1)],
                        ctb["identb"], ctb["maskT"],
                        start=True, stop=False, skip_group_check=True,
                    )
                    for cl in range(2):
                        c = 2 * cp + cl
                        nc.tensor.matmul(
                            sc[:, 1024 * X + 256 * c:1024 * X + 256 * (c + 1)],
                            ktF[64 * X:64 * (X + 1), 128 * c:128 * (c + 1)],
                            q2[64 * X:64 * (X + 1), c, :, :],
                            start=False, stop=(cl == 1), skip_group_check=True,
                        )
                nc.scalar.activation(
                    attn_u[:, 1024 * X:1024 * (X + 1)],
                    sc[:, 1024 * X:1024 * (X + 1)], Exp,
                )

            # ---- denominators: su[h, (c, bqq)] via 4 accumulating matmuls
            su = psA.tile([4, 512], F32, tag="mm512")
            attn_v4 = attn_u[:].rearrange(
                "p (x c h q) -> p x c h q", x=2, c=4, h=2)
            for h in range(4):
                nc.tensor.matmul(
                    su[:],
                    ctb["sumsI"][:, 4 * h:4 * (h + 1)],
                    attn_v4[:, h // 2, :, h % 2, :],
                    start=(h == 0), stop=(h == 3),
                )
            rec = sb.tile([4, 512], F32, tag="rec")
            nc.vector.reciprocal_approx_fast(out=rec[:], in_=su[:])
            rec_b = sb.tile([4, 512], BF16, tag="rec_b")
            nc.vector.tensor_copy(rec_b[:], rec[:])

            # ---- attn @ v, unnormalized, pair-merged
            # ytP[64X + 32hr + d, (c, hr', bqq)]; hr'!=hr halves are garbage
            ytP = psC.tile([128, 1024], F32, tag="ytP")
            for X in range(2):
                for c in range(4):
                    nc.tensor.matmul(
                        ytP[64 * X:64 * (X + 1), 256 * c:256 * (c + 1)],
                        v_s[:, 128 * c + 64 * X:128 * c + 64 * (X + 1)],
                        attn_u[:, 1024 * X + 256 * c:1024 * X + 256 * (c + 1)],
                        start=True, stop=True,
                    )

            # ---- rec broadcast over head features; one copy to SBUF
            rbP = psA.tile([128, 512], F32, tag="mm512")
            for X in range(2):
                nc.tensor.matmul(
                    rbP[64 * X:64 * (X + 1), :],
                    ctb["S4b"][0:4, 64 * X:64 * (X + 1)],
                    rec_b[:],
                    start=True, stop=True, skip_group_check=True,
                )
            rbS = sb.tile([128, 512], BF16, tag="rbS")
            nc.scalar.activation(rbS[:], rbP[:], Copy)

            # ---- extract valid blocks + normalize -> yT[128, 512] (fin, tok)
            yT = sb.tile([128, 512], BF16, tag="yT")
            ytPv = ytP[:].rearrange("p (c h q) -> p c h q", c=4, h=2)
            yTv = yT[:].rearrange("p (c q) -> p c q", c=4)
            for X in range(2):
                for hr in range(2):
                    r0 = 64 * X + 32 * hr
                    nc.vector.tensor_mul(
                        yTv[r0:r0 + 32], ytPv[r0:r0 + 32, :, hr, :],
                        rbS[r0:r0 + 32].rearrange("p (c q) -> p c q", c=4),
                    )

            pend = {"yT": yT, "m": m}

        if pend:
            emit_proj(pend)
            pend = {}
    nc.compile()
    return nc


def kernel(x, k_in, v_in, W_attn, b_attn, W_proj, b_proj):
    x = np.asarray(x, dtype=np.float32)
    cf, cf_offs, cb, cb_offs = _host_consts(
        np.asarray(W_attn, dtype=np.float32),
        np.asarray(b_attn, dtype=np.float32),
        np.asarray(W_proj, dtype=np.float32),
        np.asarray(b_proj, dtype=np.float32),
    )
    key = "prog"
    if key not in _CACHE:
        _CACHE[key] = _build_program(cf_offs, cf.shape[1], cb_offs, cb.shape[1])
    nc = _CACHE[key]

    in_maps = []
    for i in range(NCORES):
        shard = np.ascontiguousarray(x[i * BC:(i + 1) * BC].reshape(TOK, D))
        in_maps.append({"x": shard, "cf": cf, "cb": cb})

    res = run_bass_kernel_spmd(nc, in_maps, list(range(NCORES)))
    global LAST_RESULT
    LAST_RESULT = res
    outs = [res.results[i]["y"].reshape(BC, T, D) for i in range(NCORES)]
    return np.concatenate(outs, axis=0)


if __name__ == "__main__":
    rng = np.random.default_rng(0)
    xs = rng.standard_normal((B, T, D), dtype=np.float32)
    Wa = rng.standard_normal((D, 3 * D), dtype=np.float32) / np.sqrt(D)
    ba = rng.standard_normal(3 * D, dtype=np.float32) * 0.01
    Wp = rng.standard_normal((D, D), dtype=np.float32) / np.sqrt(D)
    bp = rng.standard_normal(D, dtype=np.float32) * 0.01
    out = kernel(xs, None, None, Wa, ba, Wp, bp)
    print(out.shape, out.dtype)


# revision 18
# speedup vs baseline: 1.7336x; 1.7336x over previous
"""Causal self-attention kernel for Trainium2, 8 NeuronCores, data-parallel over batch.

Problem: B=4096 independent attentions, T=64, DIM=128, 4 heads of 32; y = proj(attn(x)).
k_in / v_in inputs are unused by the module (overwritten internally) -> never shipped.

v4 "chunk-dense bf16, deep-pipelined" (per core: 512 batches = 32768 tokens,
64 mega-tiles of 512 tokens = 4 chunks of 128 tokens = 2 batches each):
  - All matmuls bf16; x is cast to bf16 on the idle GpSimd engine so the PE
    transposes run at bf16 rate.
  - q^T/k^T full-width [128,512] single matmuls. q-bias lands in the 4 strided
    PSUM->SBUF copies that build the zero-padded qt2 tiles (2 on DVE + 2 on
    ACT); k-bias dropped (softmax-invariant); v-bias folded into proj bias.
  - scores K=64 pair-packed, chunk-dense, TRANSPOSED: per (head-pair X, chunk
    c) one [64,128]x[64,256] -> [128,256] matmul. qt2[(X, hr', d), (c, hr,
    b*64+qq)] holds q with zeros on hr'!=hr (written ONCE into persistent
    tiles), so the pair contraction picks out head 2X+hr. Batches inside a
    chunk are separated by the mask seed (-80 on cross-batch and causal-
    invalid pairs): exp() kills them. attn col = 1024X + 256c + 128hr + bqq;
    PSUM bank = (X, c-pair); group = (identity-matmul mask seed + 2 scorers).
  - softmax denominators: 4 accumulating indicator matmuls -> su[4,512],
    reciprocal_approx_fast (18 bits), bf16 cast, broadcast to rbP[128,512]
    (row = feature) by 2 matmuls with out bases 0/64, one ACT copy -> SBUF.
  - attn@v UNNORMALIZED (normalization commutes with the key-sum), pair-
    merged: per (c, X) one [128,64]x[128,256] matmul -> ytP[64X+(hr,d),
    (c,hr',bqq)]; hr'!=hr halves are garbage, never read; cross-batch terms
    are ~0 through the masked attn. 4 strided DVE muls extract the valid
    blocks, normalize by rbS, and assemble yT[128,512] (fin-major) in SBUF.
  - proj natural: per chunk one K=128 matmul (lhsT = yT chunk, moving = Wp).
  - DEEP software pipeline: iteration m emits T/k/v/q(m), proj(m-2),
    softmax-tail+attn@v+normalize(m-1), scores(m), exp(m). Every PE
    instruction's cross-engine inputs are at least one mega old, so the PE
    stream runs without dependency stalls in steady state.
"""

import sys

for _p in ("/opt/trn_rl_repo", "/root/.axon_site/_ro/trn_rl_repo"):
    if _p not in sys.path:
        sys.path.insert(0, _p)

from contextlib import ExitStack

import numpy as np
import ml_dtypes

import concourse.bass as bass
import concourse.tile as tile
from concourse import bacc
from concourse import mybir
from concourse.bass_utils import run_bass_kernel_spmd

F32 = mybir.dt.float32
BF16 = mybir.dt.bfloat16
NP_BF16 = ml_dtypes.bfloat16

B, T, D, H, HS = 4096, 64, 128, 4, 32
NCORES = 8
BC = B // NCORES            # 512 batches per core
TOK = BC * T                # 32768 tokens per core
MEGA = 512                  # tokens per mega-tile (4 chunks of 128 = 8 batches)
NMEGA = TOK // MEGA         # 64
SCALE = 1.0 / float(np.sqrt(HS))
NEG = -80.0                 # additive causal mask (exp(-80) ~ 1.8e-35, harmless)

_CACHE = {}
LAST_RESULT = None


def _host_consts(W_attn, b_attn, W_proj, b_proj):
    """Precompute constant tiles on host: one fp32 pack and one bf16 pack."""
    Wq = np.ascontiguousarray(W_attn[:, 0:128] * SCALE)          # [128,128]
    Wk = np.ascontiguousarray(W_attn[:, 128:256])
    Wv = np.ascontiguousarray(W_attn[:, 256:384])
    bqs = (b_attn[0:128] * SCALE).reshape(128, 1)                # [128,1]
    bv = b_attn[256:384]
    bp_eff = b_proj + bv @ W_proj                                # [128]
    biasP = np.ascontiguousarray(
        np.broadcast_to(bp_eff.reshape(1, 1, 128), (128, 4, 128)).reshape(128, 512)
    )
    ident = np.eye(128, dtype=np.float32)
    # mask, transposed chunk-dense: rows = chunk token (b*64+kk); 512 cols of
    # 4 repeats of (b'*64+qq); 0 iff same batch and kk<=qq else NEG
    bkk = np.arange(128).reshape(128, 1)
    bqq = np.arange(128).reshape(1, 128)
    ok = ((bkk // 64) == (bqq // 64)) & ((bkk % 64) <= (bqq % 64))
    m128 = np.where(ok, 0.0, NEG).astype(np.float32)             # [128,128]
    maskT = np.ascontiguousarray(np.tile(m128, (1, 4)))          # [128,512]
    # sums stationary: sumsI[:, 4h+j] = (j==h)
    sumsI = np.zeros((128, 16), dtype=np.float32)
    for h in range(4):
        sumsI[:, 4 * h + h] = 1.0
    # bcast stationary: rb row (X, hr, d) <- rec row h=2X+hr. Two matmuls
    # (out bases 0/64): S4b[h, 64X+32hr+d] = 1, col-halves sliced per X.
    S4b = np.zeros((128, 128), dtype=np.float32)
    for h in range(4):
        c0 = 64 * (h // 2) + 32 * (h % 2)
        S4b[h, c0:c0 + 32] = 1.0

    f32_parts = [
        ("bq", bqs.astype(np.float32)),
        ("biasP", biasP.astype(np.float32)),
    ]
    bf_parts = [
        ("wq", Wq), ("wk", Wk), ("wv", Wv),
        ("wp", np.ascontiguousarray(W_proj)),
        ("identb", ident),
        ("maskT", maskT),
        ("sumsI", sumsI),
        ("S4b", S4b),
    ]

    def pack(parts, npdt):
        arrs = [np.asarray(a, dtype=np.float32).astype(npdt) for _, a in parts]
        packed = np.concatenate(arrs, axis=1)
        offs, off = {}, 0
        for (name, _), a in zip(parts, arrs):
            offs[name] = (off, a.shape[1])
            off += a.shape[1]
        return np.ascontiguousarray(packed), offs

    cf, cf_offs = pack(f32_parts, np.float32)
    cb, cb_offs = pack(bf_parts, NP_BF16)
    return cf, cf_offs, cb, cb_offs


def _build_program(cf_offs, cf_cols, cb_offs, cb_cols, ntok=TOK, stage=9):
    nmega = ntok // MEGA
    nc = bacc.Bacc()
    x_p = nc.declare_dram_parameter("x", [ntok, D], F32, isOutput=False)
    y_p = nc.declare_dram_parameter("y", [ntok, D], F32, isOutput=True)
    cf_p = nc.declare_dram_parameter("cf", [128, cf_cols], F32, isOutput=False)
    cb_p = nc.declare_dram_parameter("cb", [128, cb_cols], BF16, isOutput=False)

    # token index: t = m*512 + c*128 + p  (c = chunk, p = partition = b*64+qq)
    x_v = x_p.rearrange("(m c p) d -> m p c d", c=4, p=128)
    y_v = y_p.rearrange("(m c p) d -> m p c d", c=4, p=128)

    Copy = mybir.ActivationFunctionType.Copy
    Exp = mybir.ActivationFunctionType.Exp
    Ident = mybir.ActivationFunctionType.Identity

    with tile.TileContext(nc) as tc, ExitStack() as ctx:
        cpool = ctx.enter_context(tc.tile_pool(name="consts", bufs=1))
        sb = ctx.enter_context(tc.tile_pool(name="sb", bufs=2))
        psA = ctx.enter_context(tc.tile_pool(name="psA", bufs=2, space="PSUM"))
        psB = ctx.enter_context(tc.tile_pool(name="psB", bufs=1, space="PSUM"))
        psC = ctx.enter_context(tc.tile_pool(name="psC", bufs=1, space="PSUM"))

        callf = cpool.tile([128, cf_cols], F32, tag="c_f32")
        nc.sync.dma_start(out=callf[:], in_=cf_p[:])
        callb = cpool.tile([128, cb_cols], BF16, tag="c_bf")
        nc.sync.dma_start(out=callb[:], in_=cb_p[:])
        ctf = {n: callf[:, o:o + w] for n, (o, w) in cf_offs.items()}
        ctb = {n: callb[:, o:o + w] for n, (o, w) in cb_offs.items()}

        # persistent zero-padded q tiles for K=64 pair-packed scorers:
        # qt2[ph][(X, hr', d) row, (c, hr, bqq) col]; hr'!=hr blocks stay 0
        # (written once here), so the pair contraction picks out head 2X+hr.
        qt2 = [cpool.tile([128, 4, 2, 128], BF16, tag=f"qt2_{ph}",
                          name=f"qt2_{ph}") for ph in range(2)]
        for ph in range(2):
            for X in range(2):
                nc.vector.memset(qt2[ph][64 * X + 32:64 * X + 64, :, 0, :], 0.0)
                nc.vector.memset(qt2[ph][64 * X:64 * X + 32, :, 1, :], 0.0)

        # pipeline state: list of in-flight megas; softmax tail of m-1 and
        # proj of m-2 are emitted inside iteration m
        pend = []

        def emit_proj(p):
            yT, m = p["yT"], p["m"]
            yf = psA.tile([128, 512], F32, tag="mm512")
            for c in range(4):
                nc.tensor.matmul(
                    yf[:, c * 128:(c + 1) * 128],
                    yT[:, c * 128:(c + 1) * 128],
                    ctb["wp"],
                    start=True, stop=True,
                )
            y_out = sb.tile([128, 512], F32, tag="y_out")
            nc.vector.tensor_add(y_out[:], yf[:], ctf["biasP"])
            nc.sync.dma_start(
                out=y_v[m], in_=y_out[:].rearrange("p (c d) -> p c d", c=4)
            )

        def emit_tail(pr):
            """softmax denominators, attn@v, normalize for in-flight mega pr."""
            attn_p, v_p = pr["attn"], pr["v_s"]
            su = psA.tile([4, 512], F32, tag="mm512")
            attn_v4 = attn_p[:].rearrange(
                "p (x c h q) -> p x c h q", x=2, c=4, h=2)
            for h in range(4):
                nc.tensor.matmul(
                    su[:],
                    ctb["sumsI"][:, 4 * h:4 * (h + 1)],
                    attn_v4[:, h // 2, :, h % 2, :],
                    start=(h == 0), stop=(h == 3),
                )
            rec = sb.tile([4, 512], F32, tag="rec")
            nc.vector.reciprocal_approx_fast(out=rec[:], in_=su[:])
            rec_b = sb.tile([4, 512], BF16, tag="rec_b")
            nc.vector.tensor_copy(rec_b[:], rec[:])

            # attn @ v, unnormalized, pair-merged:
            # ytP[64X + 32hr + d, (c, hr', bqq)]; hr'!=hr halves are garbage
            ytP = psC.tile([128, 1024], F32, tag="ytP")
            for X in range(2):
                for c in range(4):
                    nc.tensor.matmul(
                        ytP[64 * X:64 * (X + 1), 256 * c:256 * (c + 1)],
                        v_p[:, 128 * c + 64 * X:128 * c + 64 * (X + 1)],
                        attn_p[:, 1024 * X + 256 * c:1024 * X + 256 * (c + 1)],
                        start=True, stop=True,
                    )

            rbP = psA.tile([128, 512], F32, tag="mm512")
            for X in range(2):
                nc.tensor.matmul(
                    rbP[64 * X:64 * (X + 1), :],
                    ctb["S4b"][0:4, 64 * X:64 * (X + 1)],
                    rec_b[:],
                    start=True, stop=True, skip_group_check=True,
                )
            rbS = sb.tile([128, 512], BF16, tag="rbS")
            nc.scalar.activation(rbS[:], rbP[:], Copy)

            yT = sb.tile([128, 512], BF16, tag="yT")
            ytPv = ytP[:].rearrange("p (c h q) -> p c h q", c=4, h=2)
            yTv = yT[:].rearrange("p (c q) -> p c q", c=4)
            for X in range(2):
                for hr in range(2):
                    r0 = 64 * X + 32 * hr
                    nc.vector.tensor_mul(
                        yTv[r0:r0 + 32], ytPv[r0:r0 + 32, :, hr, :],
                        rbS[r0:r0 + 32].rearrange("p (c q) -> p c q", c=4),
                    )
            pr["yT"] = yT

        for m in range(nmega):
            # ---- load x natural [p, c, d]; cast to bf16 on the idle GpSimd
            x_nat = sb.tile([128, 4, 128], F32, tag="x_nat")
            nc.sync.dma_start(out=x_nat[:], in_=x_v[m])
            xb = sb.tile([128, 4, 128], BF16, tag="xb")
            nc.gpsimd.tensor_copy(xb[:], x_nat[:])

            # ---- x^T via PE transpose (bf16)
            xT_ps = psA.tile([128, 512], BF16, tag="mm512")
            for c in range(4):
                nc.tensor.transpose(
                    xT_ps[:, c * 128:(c + 1) * 128], xb[:, c, :], ctb["identb"]
                )
            xT = sb.tile([128, 512], BF16, tag="xT")
            nc.scalar.activation(xT[:], xT_ps[:], Copy)

            # ---- k^T, v, q^T full-width (order chosen for PSUM slot reuse)
            k_ps = psA.tile([128, 512], F32, tag="mm512")
            nc.tensor.matmul(k_ps[:], ctb["wk"], xT[:], start=True, stop=True)
            ktF = sb.tile([128, 512], BF16, tag="ktF")
            nc.scalar.activation(ktF[:], k_ps[:], Copy)

            v_ps = psA.tile([128, 512], F32, tag="mm512")
            for c in range(4):
                nc.tensor.matmul(
                    v_ps[:, c * 128:(c + 1) * 128],
                    xT[:, c * 128:(c + 1) * 128],
                    ctb["wv"],
                    start=True, stop=True,
                )
            v_s = sb.tile([128, 512], BF16, tag="v_s")
            nc.scalar.activation(v_s[:], v_ps[:], Copy)

            q_ps = psA.tile([128, 512], F32, tag="mm512")
            nc.tensor.matmul(q_ps[:], ctb["wq"], xT[:], start=True, stop=True)
            q2 = qt2[m % 2]
            qpv = q_ps[:].rearrange("p (c q) -> p c q", c=4)
            for X in range(2):
                for hr in range(2):
                    r0 = 64 * X + 32 * hr
                    if hr == 0:  # split the bias adds across DVE and ACT
                        nc.vector.tensor_scalar_add(
                            q2[r0:r0 + 32, :, hr, :], qpv[r0:r0 + 32],
                            ctf["bq"][r0:r0 + 32, 0:1],
                        )
                    else:
                        nc.scalar.activation(
                            q2[r0:r0 + 32, :, hr, :], qpv[r0:r0 + 32],
                            Ident, bias=ctf["bq"][r0:r0 + 32, 0:1],
                        )

            # ---- proj + bias + DMA-out of mega m-2
            if len(pend) > 1:
                emit_proj(pend.pop(0))

            # ---- softmax tail + attn@v + normalize of mega m-1
            if pend:
                emit_tail(pend[-1])

            # ---- scores of mega m (inputs made this iteration), exp at tail
            # attn col = 1024X + 256c + 128hr + bqq; bank = (X, c-pair)
            sc = psB.tile([128, 2048], F32, tag="sc")
            attn_u = sb.tile([128, 2048], BF16, tag="attn")
            for X in range(2):
                for cp in range(2):
                    nc.tensor.matmul(
                        sc[:, 1024 * X + 512 * cp:1024 * X + 512 * (cp + 1)],
                        ctb["identb"], ctb["maskT"],
                        start=True, stop=False, skip_group_check=True,
                    )
                    for cl in range(2):
                        c = 2 * cp + cl
                        nc.tensor.matmul(
                            sc[:, 1024 * X + 256 * c:1024 * X + 256 * (c + 1)],
                            ktF[64 * X:64 * (X + 1), 128 * c:128 * (c + 1)],
                            q2[64 * X:64 * (X + 1), c, :, :],
                            start=False, stop=(cl == 1), skip_group_check=True,
                        )
                nc.scalar.activation(
                    attn_u[:, 1024 * X:1024 * (X + 1)],
                    sc[:, 1024 * X:1024 * (X + 1)], Exp,
                )

            pend.append({"attn": attn_u, "v_s": v_s, "m": m})

        # drain the pipeline tail: megas nmega-2 and nmega-1
        emit_tail(pend[-1])
        emit_proj(pend.pop(0))
        emit_proj(pend.pop(0))
    nc.compile()
    return nc


def kernel(x, k_in, v_in, W_attn, b_attn, W_proj, b_proj):
    x = np.asarray(x, dtype=np.float32)
    cf, cf_offs, cb, cb_offs = _host_consts(
        np.asarray(W_attn, dtype=np.float32),
        np.asarray(b_attn, dtype=np.float32),
        np.asarray(W_proj, dtype=np.float32),
        np.asarray(b_proj, dtype=np.float32),
    )
    key = "prog"
    if key not in _CACHE:
        _CACHE[key] = _build_program(cf_offs, cf.shape[1], cb_offs, cb.shape[1])
    nc = _CACHE[key]

    in_maps = []
    for i in range(NCORES):
        shard = np.ascontiguousarray(x[i * BC:(i + 1) * BC].reshape(TOK, D))
        in_maps.append({"x": shard, "cf": cf, "cb": cb})

    res = run_bass_kernel_spmd(nc, in_maps, list(range(NCORES)))
    global LAST_RESULT
    LAST_RESULT = res
    outs = [res.results[i]["y"].reshape(BC, T, D) for i in range(NCORES)]
    return np.concatenate(outs, axis=0)


if __name__ == "__main__":
    rng = np.random.default_rng(0)
    xs = rng.standard_normal((B, T, D), dtype=np.float32)
    Wa = rng.standard_normal((D, 3 * D), dtype=np.float32) / np.sqrt(D)
    ba = rng.standard_normal(3 * D, dtype=np.float32) * 0.01
    Wp = rng.standard_normal((D, D), dtype=np.float32) / np.sqrt(D)
    bp = rng.standard_normal(D, dtype=np.float32) * 0.01
    out = kernel(xs, None, None, Wa, ba, Wp, bp)
    print(out.shape, out.dtype)
